# revision 6
# baseline (speedup 1.0000x reference)
"""Trainium2 Bass kernel for nn_Concatenation_90701119357422.

Computes, for full inputs:
    ret  = mean(ret_feat, axis=1) @ Wp.T + bp          # [B, H]
    out  = concat([h, ret[batch]], -1) @ Wl.T + bl     # [N, H]

Strategy (8 cores, data-parallel over N):
  - out = h @ Wl[:, :H].T + ret2[batch]  where  ret2 = ret @ Wl[:, H:].T + bl
  - host casts h to fp16 and pre-transposes it into two feature-major halves
    per core; device runs fp16 matmuls with fp32 PSUM accumulation
  - ret2 is computed on host (tiny) and replicated as a single fp16 table
  - per-row gather ret2[batch] is a one-hot matmul accumulated into the same
    PSUM tile; the one-hot is built on device from batch values (gpsimd
    partition broadcast + DVE is_equal)
  - output is written fp16 in a feature-contiguous [128, tiles, H] layout
    (16KB DMA lines); host de-transposes and upcasts to f32
"""

import os
import sys

import numpy as np

for _p in ("/opt/trn_rl_repo", "/root/.axon_site/_ro/trn_rl_repo"):
    if os.path.isdir(_p) and _p not in sys.path:
        sys.path.append(_p)

import concourse.bass as bass
import concourse.mybir as mybir
import concourse.tile as tile
from concourse import bacc
from concourse.bass_utils import run_bass_kernel_spmd

N_TOTAL = 262144
B = 64
K = 16
H = 256
R = 512
N_CORES = 8
SHARD = N_TOTAL // N_CORES  # 32768

CHUNK = 2048                 # rows per pipeline chunk
F32 = mybir.dt.float32
F16 = mybir.dt.float16


def build_program(shard_rows: int = SHARD):
    assert shard_rows % CHUNK == 0
    n_chunks = shard_rows // CHUNK
    tiles_per_chunk = CHUNK // 128
    n_tiles_total = shard_rows // 128

    nc = bacc.Bacc("TRN2", target_bir_lowering=False, debug=False)

    # feature-major fp16 h halves: hta[k, r] = h[r, k], htb[k, r] = h[r, 128+k]
    hta_d = nc.dram_tensor("hta", [128, shard_rows], F16, kind="ExternalInput").ap()
    htb_d = nc.dram_tensor("htb", [128, shard_rows], F16, kind="ExternalInput").ap()
    bt = nc.dram_tensor("bt", [1, shard_rows], F16, kind="ExternalInput").ap()
    wt16 = nc.dram_tensor("wt16", [H, H], F16, kind="ExternalInput").ap()
    r2_d = nc.dram_tensor("r2", [128, H], F16, kind="ExternalInput").ap()
    # out_t[p, t, n] = out[128*t + p, n], fp16; host de-transposes
    out_t = nc.dram_tensor(
        "out_t", [128, n_tiles_total, H], F16, kind="ExternalOutput"
    ).ap()

    iota64_dr = nc.inline_tensor(
        np.arange(64, dtype=np.float32).reshape(64, 1), "iota64"
    ).ap()
    ones64_dr = nc.inline_tensor(
        np.ones((1, 64), dtype=np.float16), "ones64"
    ).ap()

    with tile.TileContext(nc) as tc:
        with (
            tc.tile_pool(name="const", bufs=1) as cpool,
            tc.tile_pool(name="psum", bufs=1, space="PSUM") as ppool,
            tc.tile_pool(name="ht", bufs=4) as hpool,
            tc.tile_pool(name="oh", bufs=4) as ohpool,
            tc.tile_pool(name="outp", bufs=4) as opool,
        ):
            # ---- constants into SBUF ----
            wt_sb = cpool.tile([128, 2, H], F16)
            nc.scalar.dma_start(wt_sb[:], wt16.rearrange("(kc p) c -> p kc c", p=128))
            iota64_sb = cpool.tile([64, 1], F32)
            nc.scalar.dma_start(iota64_sb[:], iota64_dr[:])
            ones64_sb = cpool.tile([1, 64], F16)
            nc.scalar.dma_start(ones64_sb[:], ones64_dr[:])
            ret2_sb = cpool.tile([64, H], F16)
            nc.scalar.dma_start(ret2_sb[:], r2_d[0:64, :])

            # ---- main loop ----
            for ci in range(n_chunks):
                r0 = ci * CHUNK
                t0 = ci * tiles_per_chunk
                bts = ohpool.tile([1, CHUNK], F16, tag="bts")
                nc.sync.dma_start(out=bts[:], in_=bt[0:1, r0 : r0 + CHUNK])
                hta = hpool.tile([128, CHUNK], F16, tag="hta")
                nc.sync.dma_start(out=hta[:], in_=hta_d[:, r0 : r0 + CHUNK])
                htb = hpool.tile([128, CHUNK], F16, tag="htb")
                nc.sync.dma_start(out=htb[:], in_=htb_d[:, r0 : r0 + CHUNK])

                # one-hot over 64 batch slots: PE rank-1 broadcast into PSUM,
                # then DVE is_equal against per-partition iota -> fp16 SBUF
                oh = ohpool.tile([64, CHUNK], F16, tag="oh")
                for half in range(CHUNK // 512):
                    hsl = slice(512 * half, 512 * (half + 1))
                    bps = ppool.tile([64, 512], F32, tag="bps", bufs=2)
                    nc.tensor.matmul(
                        bps[:], ones64_sb[:], bts[0:1, hsl], start=True, stop=True
                    )
                    nc.vector.tensor_scalar(
                        oh[:, hsl],
                        bps[:],
                        iota64_sb[:],
                        None,
                        mybir.AluOpType.is_equal,
                    )

                outsb = opool.tile([128, tiles_per_chunk, H], F16, tag="outsb")
                for t in range(tiles_per_chunk):
                    ps = ppool.tile([128, H], F32, tag="acc", bufs=6)
                    sl = slice(128 * t, 128 * (t + 1))
                    nc.tensor.matmul(
                        ps[:], hta[:, sl], wt_sb[:, 0], start=True, stop=False
                    )
                    nc.tensor.matmul(
                        ps[:], htb[:, sl], wt_sb[:, 1], start=False, stop=False
                    )
                    nc.tensor.matmul(
                        ps[:], oh[:, sl], ret2_sb[:], start=False, stop=True
                    )
                    if t % 2 == 0:
                        nc.scalar.copy(outsb[:, t], ps[:])
                    else:
                        nc.vector.tensor_copy(outsb[:, t], ps[:])

                nc.scalar.dma_start(
                    out=out_t[:, t0 : t0 + tiles_per_chunk, :], in_=outsb[:]
                )

    nc.compile()
    return nc


def prep_inputs(h, ret_feat, batch, Wp, bp, Wl, bl, shard_rows: int = SHARD,
                n_cores: int = N_CORES):
    """Host-side prep: shard + cast + pre-transpose h. Returns per-core maps."""
    h = np.asarray(h, dtype=np.float32)
    Wl = np.asarray(Wl, dtype=np.float32)
    Wp = np.asarray(Wp, dtype=np.float32)
    bp = np.asarray(bp, dtype=np.float32)
    bl = np.asarray(bl, dtype=np.float32)
    ret_feat = np.asarray(ret_feat, dtype=np.float32)

    h16 = h.astype(np.float16)
    bt_all = np.asarray(batch).astype(np.float16)

    wt16 = np.ascontiguousarray(Wl[:, :H].T).astype(np.float16)
    # replicated pooled ret table: ret2 = (mean_k rf) @ Wp.T + bp) @ Wl[:,H:].T + bl
    wlr_t = Wl[:, H:].astype(np.float64).T  # [R, H]
    ret = ret_feat.astype(np.float64).mean(axis=1) @ Wp.astype(np.float64).T + bp
    ret2 = ret @ wlr_t + bl  # [B, H] float64
    r2 = np.zeros((128, H), dtype=np.float16)
    r2[:B] = ret2.astype(np.float16)

    in_maps = []
    for i in range(n_cores):
        s = slice(i * shard_rows, (i + 1) * shard_rows)
        hs = h16[s]
        in_maps.append(
            {
                "hta": np.ascontiguousarray(hs[:, :128].T),
                "htb": np.ascontiguousarray(hs[:, 128:].T),
                "bt": np.ascontiguousarray(bt_all[s].reshape(1, shard_rows)),
                "wt16": wt16,
                "r2": r2,
            }
        )
    return in_maps


_PROGRAM_CACHE = {}


def _get_program(shard_rows: int = SHARD):
    if shard_rows not in _PROGRAM_CACHE:
        _PROGRAM_CACHE[shard_rows] = build_program(shard_rows)
    return _PROGRAM_CACHE[shard_rows]


def kernel(h, ret_feat, batch, Wp, bp, Wl, bl):
    nc = _get_program(SHARD)
    in_maps = prep_inputs(h, ret_feat, batch, Wp, bp, Wl, bl)
    res = run_bass_kernel_spmd(nc, in_maps, list(range(N_CORES)))
    outs = []
    for i in range(N_CORES):
        ot = res.results[i]["out_t"]  # [128, n_tiles, H] fp16
        outs.append(ot.transpose(1, 0, 2).reshape(SHARD, H))
    return np.concatenate(outs, axis=0).astype(np.float32)


# revision 9
# speedup vs baseline: 1.0617x; 1.0617x over previous
"""Trainium2 Bass kernel for nn_Concatenation_90701119357422.

Computes, for full inputs:
    ret  = mean(ret_feat, axis=1) @ Wp.T + bp          # [B, H]
    out  = concat([h, ret[batch]], -1) @ Wl.T + bl     # [N, H]

Strategy (8 cores, data-parallel over N):
  - out = h @ Wl[:, :H].T + ret2[batch]  where  ret2 = ret @ Wl[:, H:].T + bl
  - host casts h to fp16 and pre-transposes it into two feature-major halves
    per core; device runs fp16 matmuls with fp32 PSUM accumulation
  - ret2 is computed on host (tiny) and replicated as a single fp16 table
  - per-row gather ret2[batch] is a one-hot matmul accumulated into the same
    PSUM tile; the one-hot is built on device from batch values (gpsimd
    partition broadcast + DVE is_equal)
  - output is written fp16 in a feature-contiguous [128, tiles, H] layout
    (16KB DMA lines); host de-transposes and upcasts to f32
"""

import os
import sys

import numpy as np

for _p in ("/opt/trn_rl_repo", "/root/.axon_site/_ro/trn_rl_repo"):
    if os.path.isdir(_p) and _p not in sys.path:
        sys.path.append(_p)

import concourse.bass as bass
import concourse.mybir as mybir
import concourse.tile as tile
from concourse import bacc
from concourse.bass_utils import run_bass_kernel_spmd

N_TOTAL = 262144
B = 64
K = 16
H = 256
R = 512
N_CORES = 8
SHARD = N_TOTAL // N_CORES  # 32768

CHUNK = 4096                 # rows per pipeline chunk
F32 = mybir.dt.float32
F16 = mybir.dt.float16


def build_program(shard_rows: int = SHARD):
    assert shard_rows % CHUNK == 0
    n_chunks = shard_rows // CHUNK
    tiles_per_chunk = CHUNK // 128
    n_tiles_total = shard_rows // 128

    nc = bacc.Bacc("TRN2", target_bir_lowering=False, debug=False)

    # feature-major fp16 h halves: hta[k, r] = h[r, k], htb[k, r] = h[r, 128+k]
    hta_d = nc.dram_tensor("hta", [128, shard_rows], F16, kind="ExternalInput").ap()
    htb_d = nc.dram_tensor("htb", [128, shard_rows], F16, kind="ExternalInput").ap()
    bt = nc.dram_tensor("bt", [1, shard_rows], F16, kind="ExternalInput").ap()
    wt16 = nc.dram_tensor("wt16", [H, H], F16, kind="ExternalInput").ap()
    r2_d = nc.dram_tensor("r2", [128, H], F16, kind="ExternalInput").ap()
    # out_t[p, t, n] = out[128*t + p, n], fp16; host de-transposes
    out_t = nc.dram_tensor(
        "out_t", [128, n_tiles_total, H], F16, kind="ExternalOutput"
    ).ap()

    iota64_dr = nc.inline_tensor(
        np.arange(64, dtype=np.float32).reshape(64, 1), "iota64"
    ).ap()

    with tile.TileContext(nc) as tc:
        with (
            tc.tile_pool(name="const", bufs=1) as cpool,
            tc.tile_pool(name="psum", bufs=1, space="PSUM") as ppool,
            tc.tile_pool(name="ht", bufs=3) as hpool,
            tc.tile_pool(name="oh", bufs=3) as ohpool,
            tc.tile_pool(name="outp", bufs=3) as opool,
        ):
            # ---- constants into SBUF ----
            wt_sb = cpool.tile([128, 2, H], F16)
            nc.scalar.dma_start(wt_sb[:], wt16.rearrange("(kc p) c -> p kc c", p=128))
            iota64_sb = cpool.tile([64, 1], F32)
            nc.scalar.dma_start(iota64_sb[:], iota64_dr[:])
            ret2_sb = cpool.tile([64, H], F16)
            nc.scalar.dma_start(ret2_sb[:], r2_d[0:64, :])

            # ---- main loop ----
            for ci in range(n_chunks):
                r0 = ci * CHUNK
                t0 = ci * tiles_per_chunk
                bts = ohpool.tile([1, CHUNK], F16, tag="bts")
                nc.sync.dma_start(out=bts[:], in_=bt[0:1, r0 : r0 + CHUNK])
                hta = hpool.tile([128, CHUNK], F16, tag="hta")
                nc.sync.dma_start(out=hta[:], in_=hta_d[:, r0 : r0 + CHUNK])
                htb = hpool.tile([128, CHUNK], F16, tag="htb")
                nc.sync.dma_start(out=htb[:], in_=htb_d[:, r0 : r0 + CHUNK])

                # one-hot over 64 batch slots: Pool-engine partition broadcast,
                # then DVE is_equal against per-partition iota -> fp16 SBUF
                oh = ohpool.tile([64, CHUNK], F16, tag="oh")
                for half in range(CHUNK // 512):
                    hsl = slice(512 * half, 512 * (half + 1))
                    bcb = ohpool.tile([64, 512], F16, tag="bcb", bufs=2)
                    nc.gpsimd.partition_broadcast(bcb[:], bts[0:1, hsl])
                    nc.vector.tensor_scalar(
                        oh[:, hsl],
                        bcb[:],
                        iota64_sb[:],
                        None,
                        mybir.AluOpType.is_equal,
                    )

                outsb = opool.tile([128, tiles_per_chunk, H], F16, tag="outsb", bufs=2)
                for t in range(tiles_per_chunk):
                    ps = ppool.tile([128, H], F32, tag="acc", bufs=6)
                    sl = slice(128 * t, 128 * (t + 1))
                    nc.tensor.matmul(
                        ps[:], hta[:, sl], wt_sb[:, 0], start=True, stop=False
                    )
                    nc.tensor.matmul(
                        ps[:], htb[:, sl], wt_sb[:, 1], start=False, stop=False
                    )
                    nc.tensor.matmul(
                        ps[:], oh[:, sl], ret2_sb[:], start=False, stop=True
                    )
                    if t % 2 == 0:
                        nc.scalar.copy(outsb[:, t], ps[:])
                    else:
                        nc.vector.tensor_copy(outsb[:, t], ps[:])

                nc.scalar.dma_start(
                    out=out_t[:, t0 : t0 + tiles_per_chunk, :], in_=outsb[:]
                )

    nc.compile()
    return nc


def prep_inputs(h, ret_feat, batch, Wp, bp, Wl, bl, shard_rows: int = SHARD,
                n_cores: int = N_CORES):
    """Host-side prep: shard + cast + pre-transpose h. Returns per-core maps."""
    h = np.asarray(h, dtype=np.float32)
    Wl = np.asarray(Wl, dtype=np.float32)
    Wp = np.asarray(Wp, dtype=np.float32)
    bp = np.asarray(bp, dtype=np.float32)
    bl = np.asarray(bl, dtype=np.float32)
    ret_feat = np.asarray(ret_feat, dtype=np.float32)

    h16 = h.astype(np.float16)
    bt_all = np.asarray(batch).astype(np.float16)

    wt16 = np.ascontiguousarray(Wl[:, :H].T).astype(np.float16)
    # replicated pooled ret table: ret2 = (mean_k rf) @ Wp.T + bp) @ Wl[:,H:].T + bl
    wlr_t = Wl[:, H:].astype(np.float64).T  # [R, H]
    ret = ret_feat.astype(np.float64).mean(axis=1) @ Wp.astype(np.float64).T + bp
    ret2 = ret @ wlr_t + bl  # [B, H] float64
    r2 = np.zeros((128, H), dtype=np.float16)
    r2[:B] = ret2.astype(np.float16)

    in_maps = []
    for i in range(n_cores):
        s = slice(i * shard_rows, (i + 1) * shard_rows)
        hs = h16[s]
        in_maps.append(
            {
                "hta": np.ascontiguousarray(hs[:, :128].T),
                "htb": np.ascontiguousarray(hs[:, 128:].T),
                "bt": np.ascontiguousarray(bt_all[s].reshape(1, shard_rows)),
                "wt16": wt16,
                "r2": r2,
            }
        )
    return in_maps


_PROGRAM_CACHE = {}


def _get_program(shard_rows: int = SHARD):
    if shard_rows not in _PROGRAM_CACHE:
        _PROGRAM_CACHE[shard_rows] = build_program(shard_rows)
    return _PROGRAM_CACHE[shard_rows]


def kernel(h, ret_feat, batch, Wp, bp, Wl, bl):
    nc = _get_program(SHARD)
    in_maps = prep_inputs(h, ret_feat, batch, Wp, bp, Wl, bl)
    res = run_bass_kernel_spmd(nc, in_maps, list(range(N_CORES)))
    outs = []
    for i in range(N_CORES):
        ot = res.results[i]["out_t"]  # [128, n_tiles, H] fp16
        outs.append(ot.transpose(1, 0, 2).reshape(SHARD, H))
    return np.concatenate(outs, axis=0).astype(np.float32)


# revision 10
# speedup vs baseline: 1.0988x; 1.0349x over previous
"""Trainium2 Bass kernel for nn_Concatenation_90701119357422.

Computes, for full inputs:
    ret  = mean(ret_feat, axis=1) @ Wp.T + bp          # [B, H]
    out  = concat([h, ret[batch]], -1) @ Wl.T + bl     # [N, H]

Strategy (8 cores, data-parallel over N):
  - out = h @ Wl[:, :H].T + ret2[batch]  where  ret2 = ret @ Wl[:, H:].T + bl
  - host casts h to fp16 and pre-transposes it into two feature-major halves
    per core; device runs fp16 matmuls with fp32 PSUM accumulation
  - ret2 is computed on host (tiny) and replicated as a single fp16 table
  - per-row gather ret2[batch] is a one-hot matmul accumulated into the same
    PSUM tile; the one-hot is built on device from batch values (gpsimd
    partition broadcast + DVE is_equal)
  - output is written fp16 in a feature-contiguous [128, tiles, H] layout
    (16KB DMA lines); host de-transposes and upcasts to f32
"""

import os
import sys

import numpy as np

for _p in ("/opt/trn_rl_repo", "/root/.axon_site/_ro/trn_rl_repo"):
    if os.path.isdir(_p) and _p not in sys.path:
        sys.path.append(_p)

import concourse.bass as bass
import concourse.mybir as mybir
import concourse.tile as tile
from concourse import bacc
from concourse.bass_utils import run_bass_kernel_spmd

N_TOTAL = 262144
B = 64
K = 16
H = 256
R = 512
N_CORES = 8
SHARD = N_TOTAL // N_CORES  # 32768

CHUNK = 4096                 # rows per pipeline chunk
F32 = mybir.dt.float32
F16 = mybir.dt.float16


def build_program(shard_rows: int = SHARD):
    assert shard_rows % CHUNK == 0
    n_chunks = shard_rows // CHUNK
    tiles_per_chunk = CHUNK // 128
    n_tiles_total = shard_rows // 128

    nc = bacc.Bacc("TRN2", target_bir_lowering=False, debug=False)

    # feature-major fp16 h halves: hta[k, r] = h[r, k], htb[k, r] = h[r, 128+k]
    hta_d = nc.dram_tensor("hta", [128, shard_rows], F16, kind="ExternalInput").ap()
    htb_d = nc.dram_tensor("htb", [128, shard_rows], F16, kind="ExternalInput").ap()
    bt = nc.dram_tensor("bt", [1, shard_rows], F16, kind="ExternalInput").ap()
    wt16 = nc.dram_tensor("wt16", [H, H], F16, kind="ExternalInput").ap()
    r2_d = nc.dram_tensor("r2", [128, H], F16, kind="ExternalInput").ap()
    # out_t[p, t, n] = out[128*t + p, n], fp16; host de-transposes
    out_t = nc.dram_tensor(
        "out_t", [128, n_tiles_total, H], F16, kind="ExternalOutput"
    ).ap()

    iota64_dr = nc.inline_tensor(
        np.arange(64, dtype=np.float32).reshape(64, 1), "iota64"
    ).ap()

    with tile.TileContext(nc) as tc:
        with (
            tc.tile_pool(name="const", bufs=1) as cpool,
            tc.tile_pool(name="psum", bufs=1, space="PSUM") as ppool,
            tc.tile_pool(name="ht", bufs=3) as hpool,
            tc.tile_pool(name="oh", bufs=3) as ohpool,
            tc.tile_pool(name="outp", bufs=3) as opool,
        ):
            # ---- constants into SBUF ----
            wt_sb = cpool.tile([128, 2, H], F16)
            nc.scalar.dma_start(wt_sb[:], wt16.rearrange("(kc p) c -> p kc c", p=128))
            iota64_sb = cpool.tile([64, 1], F32)
            nc.scalar.dma_start(iota64_sb[:], iota64_dr[:])
            ret2_sb = cpool.tile([64, H], F16)
            nc.scalar.dma_start(ret2_sb[:], r2_d[0:64, :])

            # ---- main loop ----
            for ci in range(n_chunks):
                r0 = ci * CHUNK
                t0 = ci * tiles_per_chunk
                bts = ohpool.tile([1, CHUNK], F16, tag="bts")
                nc.sync.dma_start(out=bts[:], in_=bt[0:1, r0 : r0 + CHUNK])
                hta = hpool.tile([128, CHUNK], F16, tag="hta")
                nc.sync.dma_start(out=hta[:], in_=hta_d[:, r0 : r0 + CHUNK])
                htb = hpool.tile([128, CHUNK], F16, tag="htb")
                nc.sync.dma_start(out=htb[:], in_=htb_d[:, r0 : r0 + CHUNK])

                # one-hot over 64 batch slots: Pool-engine partition broadcast,
                # then DVE is_equal against per-partition iota -> fp16 SBUF
                oh = ohpool.tile([64, CHUNK], F16, tag="oh")
                for half in range(CHUNK // 512):
                    hsl = slice(512 * half, 512 * (half + 1))
                    bcb = ohpool.tile([64, 512], F16, tag="bcb", bufs=2)
                    nc.gpsimd.partition_broadcast(bcb[:], bts[0:1, hsl])
                    nc.vector.tensor_scalar(
                        oh[:, hsl],
                        bcb[:],
                        iota64_sb[:],
                        None,
                        mybir.AluOpType.is_equal,
                    )

                outsb = opool.tile([128, tiles_per_chunk, H], F16, tag="outsb", bufs=2)
                for t in range(tiles_per_chunk):
                    ps = ppool.tile([128, H], F32, tag="acc", bufs=6)
                    sl = slice(128 * t, 128 * (t + 1))
                    nc.tensor.matmul(
                        ps[:], hta[:, sl], wt_sb[:, 0], start=True, stop=False
                    )
                    nc.tensor.matmul(
                        ps[:], htb[:, sl], wt_sb[:, 1], start=False, stop=False
                    )
                    nc.tensor.matmul(
                        ps[:], oh[:, sl], ret2_sb[:], start=False, stop=True
                    )
                    nc.any.tensor_copy(outsb[:, t], ps[:])

                nc.scalar.dma_start(
                    out=out_t[:, t0 : t0 + tiles_per_chunk, :], in_=outsb[:]
                )

    nc.compile()
    return nc


def prep_inputs(h, ret_feat, batch, Wp, bp, Wl, bl, shard_rows: int = SHARD,
                n_cores: int = N_CORES):
    """Host-side prep: shard + cast + pre-transpose h. Returns per-core maps."""
    h = np.asarray(h, dtype=np.float32)
    Wl = np.asarray(Wl, dtype=np.float32)
    Wp = np.asarray(Wp, dtype=np.float32)
    bp = np.asarray(bp, dtype=np.float32)
    bl = np.asarray(bl, dtype=np.float32)
    ret_feat = np.asarray(ret_feat, dtype=np.float32)

    h16 = h.astype(np.float16)
    bt_all = np.asarray(batch).astype(np.float16)

    wt16 = np.ascontiguousarray(Wl[:, :H].T).astype(np.float16)
    # replicated pooled ret table: ret2 = (mean_k rf) @ Wp.T + bp) @ Wl[:,H:].T + bl
    wlr_t = Wl[:, H:].astype(np.float64).T  # [R, H]
    ret = ret_feat.astype(np.float64).mean(axis=1) @ Wp.astype(np.float64).T + bp
    ret2 = ret @ wlr_t + bl  # [B, H] float64
    r2 = np.zeros((128, H), dtype=np.float16)
    r2[:B] = ret2.astype(np.float16)

    in_maps = []
    for i in range(n_cores):
        s = slice(i * shard_rows, (i + 1) * shard_rows)
        hs = h16[s]
        in_maps.append(
            {
                "hta": np.ascontiguousarray(hs[:, :128].T),
                "htb": np.ascontiguousarray(hs[:, 128:].T),
                "bt": np.ascontiguousarray(bt_all[s].reshape(1, shard_rows)),
                "wt16": wt16,
                "r2": r2,
            }
        )
    return in_maps


_PROGRAM_CACHE = {}


def _get_program(shard_rows: int = SHARD):
    if shard_rows not in _PROGRAM_CACHE:
        _PROGRAM_CACHE[shard_rows] = build_program(shard_rows)
    return _PROGRAM_CACHE[shard_rows]


def kernel(h, ret_feat, batch, Wp, bp, Wl, bl):
    nc = _get_program(SHARD)
    in_maps = prep_inputs(h, ret_feat, batch, Wp, bp, Wl, bl)
    res = run_bass_kernel_spmd(nc, in_maps, list(range(N_CORES)))
    outs = []
    for i in range(N_CORES):
        ot = res.results[i]["out_t"]  # [128, n_tiles, H] fp16
        outs.append(ot.transpose(1, 0, 2).reshape(SHARD, H))
    return np.concatenate(outs, axis=0).astype(np.float32)


# revision 13
# speedup vs baseline: 1.8847x; 1.7153x over previous
"""Trainium2 Bass kernel for nn_Concatenation_90701119357422.

Computes, for full inputs:
    ret  = mean(ret_feat, axis=1) @ Wp.T + bp          # [B, H]
    out  = concat([h, ret[batch]], -1) @ Wl.T + bl     # [N, H]

Strategy (8 cores, data-parallel over N):
  - out = h @ Wl[:, :H].T + ret2[batch]  where  ret2 = ret @ Wl[:, H:].T + bl
  - host casts h to fp16 and pre-transposes it into two feature-major halves
    per core; device runs fp16 matmuls with fp32 PSUM accumulation
  - ret2 is computed on host (tiny) and replicated as a single fp16 table
  - per-row gather ret2[batch] is a one-hot matmul accumulated into the same
    PSUM tile; the one-hot is built on device from batch values (gpsimd
    partition broadcast + DVE is_equal)
  - output is written fp16 in a feature-contiguous [128, tiles, H] layout
    (16KB DMA lines); host de-transposes and upcasts to f32
"""

import os
import sys

import numpy as np

for _p in ("/opt/trn_rl_repo", "/root/.axon_site/_ro/trn_rl_repo"):
    if os.path.isdir(_p) and _p not in sys.path:
        sys.path.append(_p)

import concourse.bass as bass
import concourse.mybir as mybir
import concourse.tile as tile
from concourse import bacc
from concourse.bass_utils import run_bass_kernel_spmd

N_TOTAL = 262144
B = 64
K = 16
H = 256
R = 512
N_CORES = 8
SHARD = N_TOTAL // N_CORES  # 32768

CHUNK = 4096                 # rows per pipeline chunk
F32 = mybir.dt.float32
F16 = mybir.dt.float16


def build_program(shard_rows: int = SHARD):
    assert shard_rows % CHUNK == 0
    n_chunks = shard_rows // CHUNK
    tiles_per_chunk = CHUNK // 128
    n_tiles_total = shard_rows // 128

    nc = bacc.Bacc("TRN2", target_bir_lowering=False, debug=False)

    # feature-major fp16 h halves: hta[k, r] = h[r, k], htb[k, r] = h[r, 128+k]
    hta_d = nc.dram_tensor("hta", [128, shard_rows], F16, kind="ExternalInput").ap()
    htb_d = nc.dram_tensor("htb", [128, shard_rows], F16, kind="ExternalInput").ap()
    bt = nc.dram_tensor("bt", [1, shard_rows], F16, kind="ExternalInput").ap()
    wt16 = nc.dram_tensor("wt16", [H, H], F16, kind="ExternalInput").ap()
    r2_d = nc.dram_tensor("r2", [128, H], F16, kind="ExternalInput").ap()
    # out_t[p, t, n] = out[128*t + p, n], fp16; host de-transposes
    out_t = nc.dram_tensor(
        "out_t", [128, n_tiles_total, H], F16, kind="ExternalOutput"
    ).ap()

    iota128_dr = nc.inline_tensor(
        np.arange(128, dtype=np.float32).reshape(128, 1), "iota128"
    ).ap()

    with tile.TileContext(nc) as tc:
        with (
            tc.tile_pool(name="const", bufs=1) as cpool,
            tc.tile_pool(name="psum", bufs=1, space="PSUM") as ppool,
            tc.tile_pool(name="ht", bufs=3) as hpool,
            tc.tile_pool(name="oh", bufs=3) as ohpool,
            tc.tile_pool(name="outp", bufs=3) as opool,
        ):
            # ---- constants into SBUF ----
            wt_sb = cpool.tile([128, 2, H], F16)
            nc.scalar.dma_start(wt_sb[:], wt16.rearrange("(kc p) c -> p kc c", p=128))
            iota128_sb = cpool.tile([128, 1], F32)
            nc.scalar.dma_start(iota128_sb[:], iota128_dr[:])
            ret2_sb = cpool.tile([128, H], F16)
            nc.scalar.dma_start(ret2_sb[:], r2_d[:])

            # ---- main loop ----
            for ci in range(n_chunks):
                r0 = ci * CHUNK
                t0 = ci * tiles_per_chunk
                bts = ohpool.tile([1, CHUNK], F16, tag="bts")
                nc.sync.dma_start(out=bts[:], in_=bt[0:1, r0 : r0 + CHUNK])
                hta = hpool.tile([128, CHUNK], F16, tag="hta")
                nc.sync.dma_start(out=hta[:], in_=hta_d[:, r0 : r0 + CHUNK])
                htb = hpool.tile([128, CHUNK], F16, tag="htb")
                nc.sync.dma_start(out=htb[:], in_=htb_d[:, r0 : r0 + CHUNK])

                # one-hot: Pool-engine partition broadcast, then DVE is_equal
                # against per-partition iota -> fp16 SBUF
                oh = ohpool.tile([128, CHUNK], F16, tag="oh")
                for half in range(CHUNK // 512):
                    hsl = slice(512 * half, 512 * (half + 1))
                    bcb = ohpool.tile([128, 512], F16, tag="bcb", bufs=2)
                    nc.gpsimd.partition_broadcast(bcb[:], bts[0:1, hsl])
                    nc.vector.tensor_scalar(
                        oh[:, hsl],
                        bcb[:],
                        iota128_sb[:],
                        None,
                        mybir.AluOpType.is_equal,
                    )

                outsb = opool.tile([128, tiles_per_chunk, H], F16, tag="outsb", bufs=2)
                for t in range(tiles_per_chunk):
                    ps = ppool.tile([128, H], F32, tag="acc", bufs=6)
                    sl = slice(128 * t, 128 * (t + 1))
                    nc.tensor.matmul(
                        ps[:], hta[:, sl], wt_sb[:, 0], start=True, stop=False
                    )
                    nc.tensor.matmul(
                        ps[:], htb[:, sl], wt_sb[:, 1], start=False, stop=False
                    )
                    nc.tensor.matmul(
                        ps[:], oh[:, sl], ret2_sb[:], start=False, stop=True
                    )
                    nc.any.tensor_copy(outsb[:, t], ps[:])

                nc.scalar.dma_start(
                    out=out_t[:, t0 : t0 + tiles_per_chunk, :], in_=outsb[:]
                )

    nc.compile()
    return nc


def prep_inputs(h, ret_feat, batch, Wp, bp, Wl, bl, shard_rows: int = SHARD,
                n_cores: int = N_CORES):
    """Host-side prep: shard + cast + pre-transpose h. Returns per-core maps."""
    h = np.asarray(h, dtype=np.float32)
    Wl = np.asarray(Wl, dtype=np.float32)
    Wp = np.asarray(Wp, dtype=np.float32)
    bp = np.asarray(bp, dtype=np.float32)
    bl = np.asarray(bl, dtype=np.float32)
    ret_feat = np.asarray(ret_feat, dtype=np.float32)

    h16 = h.astype(np.float16)
    bt_all = np.asarray(batch).astype(np.float16)

    wt16 = np.ascontiguousarray(Wl[:, :H].T).astype(np.float16)
    # replicated pooled ret table: ret2 = (mean_k rf) @ Wp.T + bp) @ Wl[:,H:].T + bl
    wlr_t = Wl[:, H:].astype(np.float64).T  # [R, H]
    ret = ret_feat.astype(np.float64).mean(axis=1) @ Wp.astype(np.float64).T + bp
    ret2 = ret @ wlr_t + bl  # [B, H] float64
    r2 = np.zeros((128, H), dtype=np.float16)
    r2[:B] = ret2.astype(np.float16)

    in_maps = []
    for i in range(n_cores):
        s = slice(i * shard_rows, (i + 1) * shard_rows)
        hs = h16[s]
        in_maps.append(
            {
                "hta": np.ascontiguousarray(hs[:, :128].T),
                "htb": np.ascontiguousarray(hs[:, 128:].T),
                "bt": np.ascontiguousarray(bt_all[s].reshape(1, shard_rows)),
                "wt16": wt16,
                "r2": r2,
            }
        )
    return in_maps


_PROGRAM_CACHE = {}


def _get_program(shard_rows: int = SHARD):
    if shard_rows not in _PROGRAM_CACHE:
        _PROGRAM_CACHE[shard_rows] = build_program(shard_rows)
    return _PROGRAM_CACHE[shard_rows]


def kernel(h, ret_feat, batch, Wp, bp, Wl, bl):
    nc = _get_program(SHARD)
    in_maps = prep_inputs(h, ret_feat, batch, Wp, bp, Wl, bl)
    res = run_bass_kernel_spmd(nc, in_maps, list(range(N_CORES)))
    outs = []
    for i in range(N_CORES):
        ot = res.results[i]["out_t"]  # [128, n_tiles, H] fp16
        outs.append(ot.transpose(1, 0, 2).reshape(SHARD, H))
    return np.concatenate(outs, axis=0).astype(np.float32)


# revision 14
# speedup vs baseline: 1.9152x; 1.0162x over previous
"""Trainium2 Bass kernel for nn_Concatenation_90701119357422.

Computes, for full inputs:
    ret  = mean(ret_feat, axis=1) @ Wp.T + bp          # [B, H]
    out  = concat([h, ret[batch]], -1) @ Wl.T + bl     # [N, H]

Strategy (8 cores, data-parallel over N):
  - out = h @ Wl[:, :H].T + ret2[batch]  where  ret2 = ret @ Wl[:, H:].T + bl
  - host casts h to fp16 and pre-transposes it into two feature-major halves
    per core; device runs fp16 matmuls with fp32 PSUM accumulation
  - ret2 is computed on host (tiny) and replicated as a single fp16 table
  - per-row gather ret2[batch] is a one-hot matmul accumulated into the same
    PSUM tile; the one-hot is built on device from batch values (gpsimd
    partition broadcast + DVE is_equal)
  - output is written fp16 in a feature-contiguous [128, tiles, H] layout
    (16KB DMA lines); host de-transposes and upcasts to f32
"""

import os
import sys

import numpy as np

for _p in ("/opt/trn_rl_repo", "/root/.axon_site/_ro/trn_rl_repo"):
    if os.path.isdir(_p) and _p not in sys.path:
        sys.path.append(_p)

import concourse.bass as bass
import concourse.mybir as mybir
import concourse.tile as tile
from concourse import bacc
from concourse.bass_utils import run_bass_kernel_spmd

N_TOTAL = 262144
B = 64
K = 16
H = 256
R = 512
N_CORES = 8
SHARD = N_TOTAL // N_CORES  # 32768

CHUNK = 4096                 # rows per pipeline chunk
F32 = mybir.dt.float32
F16 = mybir.dt.float16


def build_program(shard_rows: int = SHARD):
    assert shard_rows % CHUNK == 0
    n_chunks = shard_rows // CHUNK
    tiles_per_chunk = CHUNK // 128
    n_tiles_total = shard_rows // 128

    nc = bacc.Bacc("TRN2", target_bir_lowering=False, debug=False)

    # feature-major fp16 h halves: hta[k, r] = h[r, k], htb[k, r] = h[r, 128+k]
    hta_d = nc.dram_tensor("hta", [128, shard_rows], F16, kind="ExternalInput").ap()
    htb_d = nc.dram_tensor("htb", [128, shard_rows], F16, kind="ExternalInput").ap()
    bt = nc.dram_tensor("bt", [1, shard_rows], F16, kind="ExternalInput").ap()
    wt16 = nc.dram_tensor("wt16", [H, H], F16, kind="ExternalInput").ap()
    r2_d = nc.dram_tensor("r2", [128, H], F16, kind="ExternalInput").ap()
    # out_t[p, t, n] = out[128*t + p, n], fp16; host de-transposes
    out_t = nc.dram_tensor(
        "out_t", [128, n_tiles_total, H], F16, kind="ExternalOutput"
    ).ap()

    iota128_dr = nc.inline_tensor(
        np.arange(128, dtype=np.float32).reshape(128, 1), "iota128"
    ).ap()

    with tile.TileContext(nc) as tc:
        with (
            tc.tile_pool(name="const", bufs=1) as cpool,
            tc.tile_pool(name="psum", bufs=1, space="PSUM") as ppool,
            tc.tile_pool(name="ht", bufs=3) as hpool,
            tc.tile_pool(name="oh", bufs=3) as ohpool,
            tc.tile_pool(name="outp", bufs=3) as opool,
        ):
            # ---- constants into SBUF ----
            wt_sb = cpool.tile([128, 2, H], F16)
            nc.scalar.dma_start(wt_sb[:], wt16.rearrange("(kc p) c -> p kc c", p=128))
            iota128_sb = cpool.tile([128, 1], F32)
            nc.scalar.dma_start(iota128_sb[:], iota128_dr[:])
            ret2_sb = cpool.tile([128, H], F16)
            nc.scalar.dma_start(ret2_sb[:], r2_d[:])

            # ---- main loop ----
            for ci in range(n_chunks):
                r0 = ci * CHUNK
                t0 = ci * tiles_per_chunk
                bts = ohpool.tile([1, CHUNK], F16, tag="bts")
                nc.sync.dma_start(out=bts[:], in_=bt[0:1, r0 : r0 + CHUNK])
                hta = hpool.tile([128, CHUNK], F16, tag="hta")
                nc.sync.dma_start(out=hta[:], in_=hta_d[:, r0 : r0 + CHUNK])
                htb = hpool.tile([128, CHUNK], F16, tag="htb")
                nc.sync.dma_start(out=htb[:], in_=htb_d[:, r0 : r0 + CHUNK])

                # one-hot: Pool-engine partition broadcast, then DVE is_equal
                # against per-partition iota -> fp16 SBUF
                oh = ohpool.tile([128, CHUNK], F16, tag="oh")
                for half in range(CHUNK // 512):
                    hsl = slice(512 * half, 512 * (half + 1))
                    bcb = ohpool.tile([128, 512], F16, tag="bcb", bufs=2)
                    nc.gpsimd.partition_broadcast(bcb[:], bts[0:1, hsl])
                    nc.vector.tensor_scalar(
                        oh[:, hsl],
                        bcb[:],
                        iota128_sb[:],
                        None,
                        mybir.AluOpType.is_equal,
                    )

                outsb = opool.tile([128, tiles_per_chunk, H], F16, tag="outsb", bufs=2)
                half_t = tiles_per_chunk // 2
                for t in range(tiles_per_chunk):
                    ps = ppool.tile([128, H], F32, tag="acc", bufs=8)
                    sl = slice(128 * t, 128 * (t + 1))
                    nc.tensor.matmul(
                        ps[:], hta[:, sl], wt_sb[:, 0], start=True, stop=False
                    )
                    nc.tensor.matmul(
                        ps[:], htb[:, sl], wt_sb[:, 1], start=False, stop=False
                    )
                    nc.tensor.matmul(
                        ps[:], oh[:, sl], ret2_sb[:], start=False, stop=True
                    )
                    nc.any.tensor_copy(outsb[:, t], ps[:])
                    if t == half_t - 1:
                        nc.scalar.dma_start(
                            out=out_t[:, t0 : t0 + half_t, :],
                            in_=outsb[:, 0:half_t],
                        )

                nc.scalar.dma_start(
                    out=out_t[:, t0 + half_t : t0 + tiles_per_chunk, :],
                    in_=outsb[:, half_t:tiles_per_chunk],
                )

    nc.compile()
    return nc


def prep_inputs(h, ret_feat, batch, Wp, bp, Wl, bl, shard_rows: int = SHARD,
                n_cores: int = N_CORES):
    """Host-side prep: shard + cast + pre-transpose h. Returns per-core maps."""
    h = np.asarray(h, dtype=np.float32)
    Wl = np.asarray(Wl, dtype=np.float32)
    Wp = np.asarray(Wp, dtype=np.float32)
    bp = np.asarray(bp, dtype=np.float32)
    bl = np.asarray(bl, dtype=np.float32)
    ret_feat = np.asarray(ret_feat, dtype=np.float32)

    h16 = h.astype(np.float16)
    bt_all = np.asarray(batch).astype(np.float16)

    wt16 = np.ascontiguousarray(Wl[:, :H].T).astype(np.float16)
    # replicated pooled ret table: ret2 = (mean_k rf) @ Wp.T + bp) @ Wl[:,H:].T + bl
    wlr_t = Wl[:, H:].astype(np.float64).T  # [R, H]
    ret = ret_feat.astype(np.float64).mean(axis=1) @ Wp.astype(np.float64).T + bp
    ret2 = ret @ wlr_t + bl  # [B, H] float64
    r2 = np.zeros((128, H), dtype=np.float16)
    r2[:B] = ret2.astype(np.float16)

    in_maps = []
    for i in range(n_cores):
        s = slice(i * shard_rows, (i + 1) * shard_rows)
        hs = h16[s]
        in_maps.append(
            {
                "hta": np.ascontiguousarray(hs[:, :128].T),
                "htb": np.ascontiguousarray(hs[:, 128:].T),
                "bt": np.ascontiguousarray(bt_all[s].reshape(1, shard_rows)),
                "wt16": wt16,
                "r2": r2,
            }
        )
    return in_maps


_PROGRAM_CACHE = {}


def _get_program(shard_rows: int = SHARD):
    if shard_rows not in _PROGRAM_CACHE:
        _PROGRAM_CACHE[shard_rows] = build_program(shard_rows)
    return _PROGRAM_CACHE[shard_rows]


def kernel(h, ret_feat, batch, Wp, bp, Wl, bl):
    nc = _get_program(SHARD)
    in_maps = prep_inputs(h, ret_feat, batch, Wp, bp, Wl, bl)
    res = run_bass_kernel_spmd(nc, in_maps, list(range(N_CORES)))
    outs = []
    for i in range(N_CORES):
        ot = res.results[i]["out_t"]  # [128, n_tiles, H] fp16
        outs.append(ot.transpose(1, 0, 2).reshape(SHARD, H))
    return np.concatenate(outs, axis=0).astype(np.float32)
